# revision 24
# baseline (speedup 1.0000x reference)
"""Multi-head causal attention (B=4, S=2048, D=1024, H=16) on 8 NeuronCores.

Sharding: core c -> batch b = c//2, head-group g = c%2 (8 heads each).
Each core computes, for its batch and heads:
    QT/KT = W.T @ x.T          (transposed projections, [64, S] per head)
    V     = x @ Wv             (natural layout, plus ones column for denom)
    ST    = K_chunk @ Q_blk.T  ([k=128, q<=1024] score chunks, causal-skipped)
    E     = exp(ST/8) (* triangle mask on the partial diagonal block)
    accT  = V_aug.T @ E        ([65, q]: rows 0-63 unnormalized out.T, row 64 denom)
    out.T = accT[:64] / accT[64]  (stacked over heads -> concatT [512, S])
    yT_part = W_O_part @ concatT  (output produced transposed, [D, S])
Host transposes and sums the two partial y's per batch.

HW-tuned structure (measured on this TRN2 via micro.py):
  - fp32r matmuls self-load their stationary operand (~150 ns serial per
    fresh lhsT), so attention processes 1024 q columns per head-chunk (two
    N=512 matmuls share one K/V load) and W_O runs transposed so each wot
    tile is shared by two s-block matmuls.
  - PE tile-config switches (row config 64 vs 128) cost ~0.7 us each.  The
    score matmuls therefore contract over all 128 partitions using
    persistent zero-padded KT windows (one per head parity; the unused 64
    partitions stay zero, so the other head's Q contributes exactly 0),
    matching the AV matmuls' config — zero switches in the whole phase.
  - Exps cover 1024 PSUM columns per op (ACT pays ~280 ns per PSUM-sourced
    op, so bigger ops win); pso/pp double-buffering keeps the
    score->exp->AV->normalize chain pipelined across chunks and heads.
"""

import numpy as np

import concourse.bass as bass
import concourse.tile as tile
import concourse.mybir as mybir
from concourse import bacc
from concourse.bass_utils import run_bass_kernel_spmd

B, S, D, H, HD = 4, 2048, 1024, 16, 64
NH = 8            # heads per core
NP = NH // 2      # head pairs per core
QB = 512          # matmul moving-operand block (fp32 N max)
QW = 1024         # attention q super-block (2 matmuls per weight load)
NQW = S // QW     # 2
NQB = S // QB     # 4
KC = 128          # k chunk size
NDW = QW // KC    # diagonal chunks per q super-block (8)
NKT = D // 128    # 8 contraction tiles over D
NST = S // 128    # 16 s tiles
CW = NH * HD      # 512 concat width per core

F32 = mybir.dt.float32
F32R = mybir.dt.float32r
AF = mybir.ActivationFunctionType

N_CORES = 8

_cache = {}


def _r(ap):
    return ap.bitcast(F32R)


def build_nc(repeats=1, phases="full", hw_loop=False):
    nc = bacc.Bacc("TRN2", target_bir_lowering=False, debug=False,
                   num_devices=N_CORES)
    x_ck = nc.dram_tensor("x_ck", [NQB, 128, NKT, QB], F32R,
                          kind="ExternalInput").ap()
    wq = nc.dram_tensor("wq", [128, NKT, CW], F32R, kind="ExternalInput").ap()
    wk = nc.dram_tensor("wk", [128, NKT, CW], F32R, kind="ExternalInput").ap()
    wv = nc.dram_tensor("wv", [128, NKT, CW], F32R, kind="ExternalInput").ap()
    wot = nc.dram_tensor("wot", [128, CW // 128, D], F32R,
                         kind="ExternalInput").ap()
    masks = nc.dram_tensor("masks", [KC, KC], F32R, kind="ExternalInput").ap()
    ones = nc.dram_tensor("ones", [128, NST * NH], F32R,
                          kind="ExternalInput").ap()
    yt = nc.dram_tensor("yt", [D, S], F32, kind="ExternalOutput").ap()

    with tile.TileContext(nc) as tc:
        if hw_loop:
            with tc.For_i(0, repeats, 1):
                _build(tc, x_ck, wq, wk, wv, wot, masks, ones, yt, phases)
        else:
            for _ in range(repeats):
                _build(tc, x_ck, wq, wk, wv, wot, masks, ones, yt, phases)
    nc.compile()
    return nc


def _build(tc, x_ck, wq, wk, wv, wot, masks, ones, yt, phases="full"):
    nc = tc.nc
    with tc.tile_pool(name="persist", bufs=1) as persist:
        qt_sb = persist.tile([128, NP, S], F32R)      # [2 heads, pair, s]
        kt_sb = persist.tile([128, NP, S], F32R)
        v_sb = persist.tile([128, NST, NH, HD + 1], F32R)
        tri_sb = persist.tile([128, KC], F32R)
        nc.sync.dma_start(tri_sb, masks)
        v_ones = bass.AP(tensor=v_sb.tensor, offset=v_sb.offset + HD,
                         ap=[list(v_sb.ap[0]), [HD + 1, NST * NH], [1, 1]])
        nc.sync.dma_start(v_ones, ones.rearrange("p (n o) -> p n o", o=1))

        if phases == "nop":
            with tc.tile_pool(name="nop_s", bufs=2) as nps:
                tr = nps.tile([128, KC], F32R, tag="tr")
                nc.vector.tensor_copy(tr, tri_sb)
                nc.sync.dma_start(yt[0:128, 0:KC].bitcast(F32R), tr)
            return

        # ---- projections: one streamed pass; each W k-tile loaded once per
        # ---- chunk pair (the two chunks' matmuls share the stationary)
        run_proj = phases != "dma"
        with (
            tc.tile_pool(name="pj_w", bufs=1) as pjw,
            tc.tile_pool(name="pj_x", bufs=1) as pjx,
            tc.tile_pool(name="pj_p", bufs=1, space="PSUM") as pjp,
        ):
            wq_sb = pjw.tile([128, NKT, CW], F32R)
            wk_sb = pjw.tile([128, NKT, CW], F32R)
            wv_sb = pjw.tile([128, NKT, CW], F32R)
            # weight DMAs on the ACT HWDGE ring; x chunks ride the SP ring
            nc.scalar.dma_start(wq_sb, wq)
            nc.scalar.dma_start(wk_sb, wk)
            nc.scalar.dma_start(wv_sb, wv)
            for cp in range(2):
                # chunk pair: each W k-tile stationary is shared by the two
                # chunks' matmuls (fresh lhsT ~408 ns vs shared ~260 ns);
                # 3-slot xs rotation keeps the next chunk's DMA prefetched
                c0, c1 = 2 * cp, 2 * cp + 1
                xsA = pjx.tile([128, NKT, QB], F32R, tag="xs0",
                               name=f"xsA{cp}")
                xsB = pjx.tile([128, NKT, QB], F32R, tag="xs1",
                               name=f"xsB{cp}")
                nc.sync.dma_start(xsA, x_ck[c0])
                nc.sync.dma_start(xsB, x_ck[c1])
                if not run_proj:
                    continue
                slA = slice(c0 * QB, (c0 + 1) * QB)
                slB = slice(c1 * QB, (c1 + 1) * QB)
                for p in range(NP):
                    tA = pjp.tile([128, QB], F32, tag=f"pt{p % 2}",
                                  name=f"qA{cp}{p}")
                    tB = pjp.tile([128, QB], F32, tag=f"pt{2 + p % 2}",
                                  name=f"qB{cp}{p}")
                    for k in range(NKT):
                        lhs = _r(wq_sb[:, k, p * 128:(p + 1) * 128])
                        nc.tensor.matmul(tA, lhs, _r(xsA[:, k, :]),
                                         start=(k == 0), stop=(k == NKT - 1))
                        nc.tensor.matmul(tB, lhs, _r(xsB[:, k, :]),
                                         start=(k == 0), stop=(k == NKT - 1))
                    nc.vector.tensor_copy(qt_sb[:, p, slA], tA)
                    nc.vector.tensor_copy(qt_sb[:, p, slB], tB)
                for p in range(NP):
                    tA = pjp.tile([128, QB], F32, tag=f"pt{4 + p % 2}",
                                  name=f"kA{cp}{p}")
                    tB = pjp.tile([128, QB], F32, tag=f"pt{6 + p % 2}",
                                  name=f"kB{cp}{p}")
                    for k in range(NKT):
                        lhs = _r(wk_sb[:, k, p * 128:(p + 1) * 128])
                        nc.tensor.matmul(tA, lhs, _r(xsA[:, k, :]),
                                         start=(k == 0), stop=(k == NKT - 1))
                        nc.tensor.matmul(tB, lhs, _r(xsB[:, k, :]),
                                         start=(k == 0), stop=(k == NKT - 1))
                    nc.scalar.copy(kt_sb[:, p, slA], tA)
                    nc.scalar.copy(kt_sb[:, p, slB], tB)
                for vi, (ci, xsX, slX) in enumerate(
                        ((c0, xsA, slA), (c1, xsB, slB))):
                    for i in range(4):
                        tv = pjp.tile([128, QB], F32,
                                      tag=f"pt{(4 * vi + i) % 8}",
                                      name=f"v{ci}{i}")
                        for k in range(NKT):
                            nc.tensor.matmul(
                                tv, _r(xsX[:, k, i * 128:(i + 1) * 128]),
                                _r(wv_sb[:, k, :]),
                                start=(k == 0), stop=(k == NKT - 1))
                        nc.vector.tensor_copy(
                            v_sb[:, ci * 4 + i, :, 0:HD],
                            tv.rearrange("p (h e) -> p h e", h=NH))

        with tc.tile_pool(name="wo_w", bufs=1) as wow_pool:
            _build_tail(tc, wow_pool, qt_sb, kt_sb, v_sb, tri_sb, persist,
                        wot, yt, phases)


def _build_tail(tc, wow_pool, qt_sb, kt_sb, v_sb, tri_sb, persist, wot, yt,
                phases):
        nc = tc.nc
        wot_sb = wow_pool.tile([128, CW // 128, D], F32R)
        nc.scalar.dma_start(wot_sb, wot)

        if phases == "dma":
            with tc.tile_pool(name="dma_s", bufs=2) as dms:
                for t in range(8):
                    for half in range(2):
                        ysb = dms.tile([128, 1024], F32R, tag="ysb")
                        nc.vector.tensor_copy(
                            ysb, wot_sb[:, t % 4, :].rearrange("p d -> p d"))
                        nc.sync.dma_start(
                            yt[t * 128:(t + 1) * 128,
                               half * 1024:(half + 1) * 1024].bitcast(F32R),
                            ysb)
            return

        if phases in ("vqk", "qk", "v"):
            # truncated build for HW bisection: write qt/kt straight out
            with tc.tile_pool(name="tr_s", bufs=2) as trs:
                for c in range(NQB):
                    tr = trs.tile([128, QB], F32R, tag="tr")
                    if phases == "v":
                        nc.vector.tensor_copy(
                            tr, v_sb[:, c, :, :].rearrange(
                                "p h e -> p (h e)")[:, 0:QB])
                    else:
                        nc.vector.tensor_mul(
                            tr, qt_sb[:, 0, c * QB:(c + 1) * QB],
                            kt_sb[:, 0, c * QB:(c + 1) * QB])
                    nc.sync.dma_start(
                        yt[c * 128:(c + 1) * 128, 0:QB].bitcast(F32R), tr)
            return

        # ---- attention: 1024 q per head-chunk, shared K/V weight loads.
        # ---- Chunks processed in 8-chunk sweeps: all score matmuls (one PE
        # ---- tile config), then drains (exp via ACT/DVE split), then all
        # ---- AV matmuls (the other config) — tile-config switches cost
        # ---- ~0.7us each on HW, so batching them matters.
        with (
            tc.tile_pool(name="at_c", bufs=1) as atc,
            tc.tile_pool(name="at_e", bufs=7) as ate,
            tc.tile_pool(name="at_r", bufs=1) as atr,
            tc.tile_pool(name="wo_s", bufs=2) as wos,
            tc.tile_pool(name="at_pp", bufs=2, space="PSUM") as atpp,
            tc.tile_pool(name="at_po", bufs=2, space="PSUM") as atpo,
        ):
            concat_sb = atc.tile([128, NP, S], F32R)
            # zero-padded KT windows (one per head parity): rows outside the
            # head's 64 partitions stay zero, so score matmuls can contract
            # over all 128 partitions — same PE tile config as the AV
            # matmuls (config switches cost ~0.7us each on HW)
            ktp = [persist.tile([128, S], F32R, name=f"ktp{i}")
                   for i in range(2)]
            nc.vector.memset(ktp[0].bitcast(F32)[64:128, :], 0.0)
            nc.vector.memset(ktp[1].bitcast(F32)[0:64, :], 0.0)
            for qw in range(NQW):
                w0 = qw * QW
                nfull = qw * NDW            # full chunks below this block
                nkc = nfull + NDW
                for h in range(NH):
                    p, r0 = h // 2, 64 * (h % 2)
                    ktw = ktp[h % 2]
                    nc.vector.tensor_copy(
                        ktw[r0:r0 + 64, 0:nkc * KC],
                        kt_sb[r0:r0 + 64, p, 0:nkc * KC])
                    pso = atpo.tile([HD + 1, 2, QB], F32, tag="pso")
                    pso_f = pso.rearrange("p a b -> p (a b)")
                    for kc in range(nkc):
                        ksl = slice(kc * KC, (kc + 1) * KC)
                        lhs = _r(ktw[:, ksl])
                        q0 = (kc - nfull) * KC if kc >= nfull else 0
                        pp = atpp.tile([128, 2, QB], F32, tag="pp")
                        pp_f = pp.rearrange("p a b -> p (a b)")
                        if q0 < QB:
                            nc.tensor.matmul(
                                pp_f[:, q0:QB], lhs,
                                _r(qt_sb[:, p, w0 + q0:w0 + QB]),
                                start=True, stop=True)
                            nc.tensor.matmul(
                                pp_f[:, QB:], lhs,
                                _r(qt_sb[:, p, w0 + QB:w0 + QW]),
                                start=True, stop=True)
                        else:
                            nc.tensor.matmul(
                                pp_f[:, q0:], lhs,
                                _r(qt_sb[:, p, w0 + q0:w0 + QW]),
                                start=True, stop=True)
                        ex = ate.tile([128, 2, QB], F32R, tag="ex")
                        ex_f = ex.rearrange("p a b -> p (a b)")
                        nc.scalar.activation(ex_f[:, q0:], pp_f[:, q0:],
                                             AF.Exp, scale=0.125)
                        if kc >= nfull:
                            nc.vector.tensor_mul(ex_f[:, q0:q0 + KC],
                                                 ex_f[:, q0:q0 + KC],
                                                 tri_sb)
                        j = kc - nfull
                        stop0 = (j == (QB // KC) - 1)
                        stop1 = (j == NDW - 1)
                        vlhs = _r(v_sb[:, kc, h, :])
                        if q0 == 0:
                            nc.tensor.matmul(
                                pso[:, 0, :], vlhs, _r(ex[:, 0, :]),
                                start=(kc == 0), stop=stop0)
                            nc.tensor.matmul(
                                pso[:, 1, :], vlhs, _r(ex[:, 1, :]),
                                start=(kc == 0), stop=stop1)
                        elif q0 < QB:
                            nc.tensor.matmul(
                                pso[:, 0, q0:], vlhs, _r(ex[:, 0, q0:]),
                                start=False, stop=stop0)
                            nc.tensor.matmul(
                                pso[:, 1, :], vlhs, _r(ex[:, 1, :]),
                                start=False, stop=stop1)
                        else:
                            nc.tensor.matmul(
                                pso[:, 1, q0 - QB:], vlhs,
                                _r(ex[:, 1, q0 - QB:]),
                                start=False, stop=stop1)
                    # normalize: divide rows 0-63 by the denominator row
                    recip = atr.tile([1, QW], F32, tag="recip")
                    nc.vector.reciprocal(recip, pso_f[HD:HD + 1, :])
                    recip_b = atr.tile([64, QW], F32, tag="recip_b")
                    nc.gpsimd.partition_broadcast(recip_b, recip)
                    nc.vector.tensor_mul(
                        concat_sb[r0:r0 + 64, p, w0:w0 + QW],
                        pso_f[0:HD, :], recip_b)

                # W_O transposed: yT[d, s] = sum_c wot[c, d] * concat[c, s];
                # each wot tile is stationary for two N=512 matmuls
                for dt_ in range(D // 128):
                    dsl = slice(dt_ * 128, (dt_ + 1) * 128)
                    ysb = wos.tile([128, QW], F32, tag="ysb")
                    psy = atpp.tile([128, 2, QB], F32, tag="pp", name="psy")
                    for cc in range(CW // 128):
                        lhs = _r(wot_sb[:, cc, dsl])
                        nc.tensor.matmul(
                            psy[:, 0, :], lhs,
                            _r(concat_sb[:, cc, w0:w0 + QB]),
                            start=(cc == 0), stop=(cc == CW // 128 - 1))
                        nc.tensor.matmul(
                            psy[:, 1, :], lhs,
                            _r(concat_sb[:, cc, w0 + QB:w0 + QW]),
                            start=(cc == 0), stop=(cc == CW // 128 - 1))
                    nc.vector.tensor_copy(
                        ysb, psy.rearrange("p a b -> p (a b)"))
                    nc.sync.dma_start(yt[dsl, w0:w0 + QW], ysb)


def shard_inputs(x, Wq, Wk, Wv, W_O):
    """Build the 8 per-core input maps from full inputs."""
    masks = (np.arange(KC)[:, None] <= np.arange(KC)[None, :]).astype(
        np.float32)

    def wtile(w):
        # [D, CW] -> [128, NKT, CW] with row d = k*128 + p
        return np.ascontiguousarray(w.reshape(NKT, 128, CW).transpose(1, 0, 2))

    in_maps = []
    for c in range(N_CORES):
        b, g = c // 2, c % 2
        hs = slice(g * NH, (g + 1) * NH)
        xT = np.ascontiguousarray(x[b].T)
        x_ck = np.ascontiguousarray(
            xT.reshape(NKT, 128, NQB, QB).transpose(2, 1, 0, 3))
        wot = np.ascontiguousarray(W_O[:, g * CW:(g + 1) * CW].T)
        in_maps.append({
            "x_ck": x_ck,
            "wq": wtile(Wq[hs].transpose(1, 0, 2).reshape(D, CW)),
            "wk": wtile(Wk[hs].transpose(1, 0, 2).reshape(D, CW)),
            "wv": wtile(Wv[hs].transpose(1, 0, 2).reshape(D, CW)),
            "wot": np.ascontiguousarray(
                wot.reshape(CW // 128, 128, D).transpose(1, 0, 2)),
            "masks": masks,
            "ones": np.ones((128, NST * NH), np.float32),
        })
    return in_maps


def kernel(x, Wq, Wk, Wv, W_O):
    x = np.asarray(x, np.float32)
    Wq = np.asarray(Wq, np.float32)
    Wk = np.asarray(Wk, np.float32)
    Wv = np.asarray(Wv, np.float32)
    W_O = np.asarray(W_O, np.float32)

    if "nc" not in _cache:
        _cache["nc"] = build_nc()
    nc = _cache["nc"]

    in_maps = shard_inputs(x, Wq, Wk, Wv, W_O)
    res = run_bass_kernel_spmd(nc, in_maps, core_ids=list(range(N_CORES)))
    _cache["last_results"] = res

    y = np.zeros((B, S, D), np.float32)
    for c in range(N_CORES):
        y[c // 2] += res.results[c]["yt"].T
    return y


# revision 25
# speedup vs baseline: 1.1587x; 1.1587x over previous
"""Multi-head causal attention (B=4, S=2048, D=1024, H=16) on 8 NeuronCores.

Sharding: core c -> batch b = c//2, head-group g = c%2 (8 heads each).
Each core computes, for its batch and heads:
    QT/KT = W.T @ x.T          (transposed projections, [64, S] per head)
    V     = x @ Wv             (natural layout, plus ones column for denom)
    ST    = K_chunk @ Q_blk.T  ([k=128, q<=1024] score chunks, causal-skipped)
    E     = exp(ST/8) (* triangle mask on the partial diagonal block)
    accT  = V_aug.T @ E        ([65, q]: rows 0-63 unnormalized out.T, row 64 denom)
    out.T = accT[:64] / accT[64]  (stacked over heads -> concatT [512, S])
    yT_part = W_O_part @ concatT  (output produced transposed, [D, S])
Host transposes and sums the two partial y's per batch.

HW-tuned structure (measured on this TRN2 via micro.py):
  - fp32r matmuls self-load their stationary operand (~150 ns serial per
    fresh lhsT), so attention processes 1024 q columns per head-chunk (two
    N=512 matmuls share one K/V load) and W_O runs transposed so each wot
    tile is shared by two s-block matmuls.
  - PE tile-config switches (row config 64 vs 128) cost ~0.7 us each.  The
    score matmuls therefore contract over all 128 partitions using
    persistent zero-padded KT windows (one per head parity; the unused 64
    partitions stay zero, so the other head's Q contributes exactly 0),
    matching the AV matmuls' config — zero switches in the whole phase.
  - Exps cover 1024 PSUM columns per op (ACT pays ~280 ns per PSUM-sourced
    op, so bigger ops win); pso/pp double-buffering keeps the
    score->exp->AV->normalize chain pipelined across chunks and heads.
"""

import numpy as np

import concourse.bass as bass
import concourse.tile as tile
import concourse.mybir as mybir
from concourse import bacc
from concourse.bass_utils import run_bass_kernel_spmd

B, S, D, H, HD = 4, 2048, 1024, 16, 64
NH = 8            # heads per core
NP = NH // 2      # head pairs per core
QB = 512          # matmul moving-operand block (fp32 N max)
QW = 1024         # attention q super-block (2 matmuls per weight load)
NQW = S // QW     # 2
NQB = S // QB     # 4
KC = 128          # k chunk size
NDW = QW // KC    # diagonal chunks per q super-block (8)
NKT = D // 128    # 8 contraction tiles over D
NST = S // 128    # 16 s tiles
CW = NH * HD      # 512 concat width per core

F32 = mybir.dt.float32
F32R = mybir.dt.float32r
AF = mybir.ActivationFunctionType

N_CORES = 8

_cache = {}


def _r(ap):
    return ap.bitcast(F32R)


def build_nc(repeats=1, phases="full", hw_loop=False):
    nc = bacc.Bacc("TRN2", target_bir_lowering=False, debug=False,
                   num_devices=N_CORES)
    x_ck = nc.dram_tensor("x_ck", [NQB, 128, NKT, QB], F32R,
                          kind="ExternalInput").ap()
    wq = nc.dram_tensor("wq", [128, NKT, CW], F32R, kind="ExternalInput").ap()
    wk = nc.dram_tensor("wk", [128, NKT, CW], F32R, kind="ExternalInput").ap()
    wv = nc.dram_tensor("wv", [128, NKT, CW], F32R, kind="ExternalInput").ap()
    wot = nc.dram_tensor("wot", [128, CW // 128, D], F32R,
                         kind="ExternalInput").ap()
    masks = nc.dram_tensor("masks", [KC, KC], F32R, kind="ExternalInput").ap()
    ones = nc.dram_tensor("ones", [128, NST * NH], F32R,
                          kind="ExternalInput").ap()
    yt = nc.dram_tensor("yt", [D, S], F32, kind="ExternalOutput").ap()

    with tile.TileContext(nc) as tc:
        if hw_loop:
            with tc.For_i(0, repeats, 1):
                _build(tc, x_ck, wq, wk, wv, wot, masks, ones, yt, phases)
        else:
            for _ in range(repeats):
                _build(tc, x_ck, wq, wk, wv, wot, masks, ones, yt, phases)
    nc.compile()
    return nc


def _build(tc, x_ck, wq, wk, wv, wot, masks, ones, yt, phases="full"):
    nc = tc.nc
    with tc.tile_pool(name="persist", bufs=1) as persist:
        qt_sb = persist.tile([128, NP, S], F32R)      # [2 heads, pair, s]
        kt_sb = persist.tile([128, NP, S], F32R)
        v_sb = persist.tile([128, NST, NH, HD + 1], F32R)
        tri_sb = persist.tile([128, KC], F32R)
        nc.sync.dma_start(tri_sb, masks)
        v_ones = bass.AP(tensor=v_sb.tensor, offset=v_sb.offset + HD,
                         ap=[list(v_sb.ap[0]), [HD + 1, NST * NH], [1, 1]])
        nc.sync.dma_start(v_ones, ones.rearrange("p (n o) -> p n o", o=1))

        if phases == "nop":
            with tc.tile_pool(name="nop_s", bufs=2) as nps:
                tr = nps.tile([128, KC], F32R, tag="tr")
                nc.vector.tensor_copy(tr, tri_sb)
                nc.sync.dma_start(yt[0:128, 0:KC].bitcast(F32R), tr)
            return

        # ---- projections: one streamed pass; each W k-tile loaded once per
        # ---- chunk pair (the two chunks' matmuls share the stationary)
        run_proj = phases != "dma"
        with (
            tc.tile_pool(name="pj_w", bufs=1) as pjw,
            tc.tile_pool(name="pj_x", bufs=1) as pjx,
            tc.tile_pool(name="pj_p", bufs=1, space="PSUM") as pjp,
        ):
            wq_sb = pjw.tile([128, NKT, CW], F32R)
            wk_sb = pjw.tile([128, NKT, CW], F32R)
            wv_sb = pjw.tile([128, NKT, CW], F32R)
            # weight DMAs on the ACT HWDGE ring; x chunks ride the SP ring
            nc.scalar.dma_start(wq_sb, wq)
            nc.scalar.dma_start(wk_sb, wk)
            nc.scalar.dma_start(wv_sb, wv)
            for c in range(NQB):
                csl = slice(c * QB, (c + 1) * QB)
                xs = pjx.tile([128, NKT, QB], F32R, tag=f"xs{c % 2}")
                nc.sync.dma_start(xs, x_ck[c])
                if not run_proj:
                    continue
                for p in range(NP):
                    ps = pjp.tile([128, QB], F32, tag=f"pt{p % 2}",
                                  name=f"q{c}{p}")
                    for k in range(NKT):
                        nc.tensor.matmul(
                            ps, _r(wq_sb[:, k, p * 128:(p + 1) * 128]),
                            _r(xs[:, k, :]),
                            start=(k == 0), stop=(k == NKT - 1))
                    nc.vector.tensor_copy(qt_sb[:, p, csl], ps)
                for p in range(NP):
                    ps = pjp.tile([128, QB], F32, tag=f"pt{2 + p % 2}",
                                  name=f"k{c}{p}")
                    for k in range(NKT):
                        nc.tensor.matmul(
                            ps, _r(wk_sb[:, k, p * 128:(p + 1) * 128]),
                            _r(xs[:, k, :]),
                            start=(k == 0), stop=(k == NKT - 1))
                    nc.scalar.copy(kt_sb[:, p, csl], ps)
                for i in range(4):
                    ps = pjp.tile([128, QB], F32, tag=f"pt{4 + i % 4}",
                                  name=f"v{c}{i}")
                    for k in range(NKT):
                        nc.tensor.matmul(
                            ps, _r(xs[:, k, i * 128:(i + 1) * 128]),
                            _r(wv_sb[:, k, :]),
                            start=(k == 0), stop=(k == NKT - 1))
                    nc.vector.tensor_copy(
                        v_sb[:, c * 4 + i, :, 0:HD],
                        ps.rearrange("p (h e) -> p h e", h=NH))

        with tc.tile_pool(name="wo_w", bufs=1) as wow_pool:
            _build_tail(tc, wow_pool, qt_sb, kt_sb, v_sb, tri_sb, persist,
                        wot, yt, phases)


def _build_tail(tc, wow_pool, qt_sb, kt_sb, v_sb, tri_sb, persist, wot, yt,
                phases):
        nc = tc.nc
        wot_sb = wow_pool.tile([128, CW // 128, D], F32R)
        nc.scalar.dma_start(wot_sb, wot)

        if phases == "dma":
            with tc.tile_pool(name="dma_s", bufs=2) as dms:
                for t in range(8):
                    for half in range(2):
                        ysb = dms.tile([128, 1024], F32R, tag="ysb")
                        nc.vector.tensor_copy(
                            ysb, wot_sb[:, t % 4, :].rearrange("p d -> p d"))
                        nc.sync.dma_start(
                            yt[t * 128:(t + 1) * 128,
                               half * 1024:(half + 1) * 1024].bitcast(F32R),
                            ysb)
            return

        if phases in ("vqk", "qk", "v"):
            # truncated build for HW bisection: write qt/kt straight out
            with tc.tile_pool(name="tr_s", bufs=2) as trs:
                for c in range(NQB):
                    tr = trs.tile([128, QB], F32R, tag="tr")
                    if phases == "v":
                        nc.vector.tensor_copy(
                            tr, v_sb[:, c, :, :].rearrange(
                                "p h e -> p (h e)")[:, 0:QB])
                    else:
                        nc.vector.tensor_mul(
                            tr, qt_sb[:, 0, c * QB:(c + 1) * QB],
                            kt_sb[:, 0, c * QB:(c + 1) * QB])
                    nc.sync.dma_start(
                        yt[c * 128:(c + 1) * 128, 0:QB].bitcast(F32R), tr)
            return

        # ---- attention: 1024 q per head-chunk, shared K/V weight loads.
        # ---- Chunks processed in 8-chunk sweeps: all score matmuls (one PE
        # ---- tile config), then drains (exp via ACT/DVE split), then all
        # ---- AV matmuls (the other config) — tile-config switches cost
        # ---- ~0.7us each on HW, so batching them matters.
        with (
            tc.tile_pool(name="at_c", bufs=1) as atc,
            tc.tile_pool(name="at_e", bufs=7) as ate,
            tc.tile_pool(name="at_r", bufs=1) as atr,
            tc.tile_pool(name="wo_s", bufs=2) as wos,
            tc.tile_pool(name="at_pp", bufs=2, space="PSUM") as atpp,
            tc.tile_pool(name="at_po", bufs=2, space="PSUM") as atpo,
        ):
            concat_sb = atc.tile([128, NP, S], F32R)
            # zero-padded KT windows (one per head parity): rows outside the
            # head's 64 partitions stay zero, so score matmuls can contract
            # over all 128 partitions — same PE tile config as the AV
            # matmuls (config switches cost ~0.7us each on HW)
            ktp = [persist.tile([128, S], F32R, name=f"ktp{i}")
                   for i in range(2)]
            nc.vector.memset(ktp[0].bitcast(F32)[64:128, :], 0.0)
            nc.vector.memset(ktp[1].bitcast(F32)[0:64, :], 0.0)
            for qw in range(NQW):
                w0 = qw * QW
                nfull = qw * NDW            # full chunks below this block
                nkc = nfull + NDW
                for h in range(NH):
                    p, r0 = h // 2, 64 * (h % 2)
                    ktw = ktp[h % 2]
                    nc.vector.tensor_copy(
                        ktw[r0:r0 + 64, 0:nkc * KC],
                        kt_sb[r0:r0 + 64, p, 0:nkc * KC])
                    pso = atpo.tile([HD + 1, 2, QB], F32, tag="pso")
                    pso_f = pso.rearrange("p a b -> p (a b)")
                    for kc in range(nkc):
                        ksl = slice(kc * KC, (kc + 1) * KC)
                        lhs = _r(ktw[:, ksl])
                        q0 = (kc - nfull) * KC if kc >= nfull else 0
                        pp = atpp.tile([128, 2, QB], F32, tag="pp")
                        pp_f = pp.rearrange("p a b -> p (a b)")
                        if q0 < QB:
                            nc.tensor.matmul(
                                pp_f[:, q0:QB], lhs,
                                _r(qt_sb[:, p, w0 + q0:w0 + QB]),
                                start=True, stop=True)
                            nc.tensor.matmul(
                                pp_f[:, QB:], lhs,
                                _r(qt_sb[:, p, w0 + QB:w0 + QW]),
                                start=True, stop=True)
                        else:
                            nc.tensor.matmul(
                                pp_f[:, q0:], lhs,
                                _r(qt_sb[:, p, w0 + q0:w0 + QW]),
                                start=True, stop=True)
                        ex = ate.tile([128, 2, QB], F32R, tag="ex")
                        ex_f = ex.rearrange("p a b -> p (a b)")
                        nc.scalar.activation(ex_f[:, q0:], pp_f[:, q0:],
                                             AF.Exp, scale=0.125)
                        if kc >= nfull:
                            nc.vector.tensor_mul(ex_f[:, q0:q0 + KC],
                                                 ex_f[:, q0:q0 + KC],
                                                 tri_sb)
                        j = kc - nfull
                        stop0 = (j == (QB // KC) - 1)
                        stop1 = (j == NDW - 1)
                        vlhs = _r(v_sb[:, kc, h, :])
                        if q0 == 0:
                            nc.tensor.matmul(
                                pso[:, 0, :], vlhs, _r(ex[:, 0, :]),
                                start=(kc == 0), stop=stop0)
                            nc.tensor.matmul(
                                pso[:, 1, :], vlhs, _r(ex[:, 1, :]),
                                start=(kc == 0), stop=stop1)
                        elif q0 < QB:
                            nc.tensor.matmul(
                                pso[:, 0, q0:], vlhs, _r(ex[:, 0, q0:]),
                                start=False, stop=stop0)
                            nc.tensor.matmul(
                                pso[:, 1, :], vlhs, _r(ex[:, 1, :]),
                                start=False, stop=stop1)
                        else:
                            nc.tensor.matmul(
                                pso[:, 1, q0 - QB:], vlhs,
                                _r(ex[:, 1, q0 - QB:]),
                                start=False, stop=stop1)
                    # normalize: divide rows 0-63 by the denominator row
                    recip = atr.tile([1, QW], F32, tag="recip")
                    nc.vector.reciprocal(recip, pso_f[HD:HD + 1, :])
                    recip_b = atr.tile([64, QW], F32, tag="recip_b")
                    nc.gpsimd.partition_broadcast(recip_b, recip)
                    nc.vector.tensor_mul(
                        concat_sb[r0:r0 + 64, p, w0:w0 + QW],
                        pso_f[0:HD, :], recip_b)

                # W_O transposed: yT[d, s] = sum_c wot[c, d] * concat[c, s];
                # each wot tile is stationary for two N=512 matmuls
                for dt_ in range(D // 128):
                    dsl = slice(dt_ * 128, (dt_ + 1) * 128)
                    ysb = wos.tile([128, QW], F32, tag="ysb")
                    psy = atpp.tile([128, 2, QB], F32, tag="pp", name="psy")
                    for cc in range(CW // 128):
                        lhs = _r(wot_sb[:, cc, dsl])
                        nc.tensor.matmul(
                            psy[:, 0, :], lhs,
                            _r(concat_sb[:, cc, w0:w0 + QB]),
                            start=(cc == 0), stop=(cc == CW // 128 - 1))
                        nc.tensor.matmul(
                            psy[:, 1, :], lhs,
                            _r(concat_sb[:, cc, w0 + QB:w0 + QW]),
                            start=(cc == 0), stop=(cc == CW // 128 - 1))
                    nc.vector.tensor_copy(
                        ysb, psy.rearrange("p a b -> p (a b)"))
                    nc.sync.dma_start(yt[dsl, w0:w0 + QW], ysb)


def shard_inputs(x, Wq, Wk, Wv, W_O):
    """Build the 8 per-core input maps from full inputs."""
    masks = (np.arange(KC)[:, None] <= np.arange(KC)[None, :]).astype(
        np.float32)

    def wtile(w):
        # [D, CW] -> [128, NKT, CW] with row d = k*128 + p
        return np.ascontiguousarray(w.reshape(NKT, 128, CW).transpose(1, 0, 2))

    in_maps = []
    for c in range(N_CORES):
        b, g = c // 2, c % 2
        hs = slice(g * NH, (g + 1) * NH)
        xT = np.ascontiguousarray(x[b].T)
        x_ck = np.ascontiguousarray(
            xT.reshape(NKT, 128, NQB, QB).transpose(2, 1, 0, 3))
        wot = np.ascontiguousarray(W_O[:, g * CW:(g + 1) * CW].T)
        in_maps.append({
            "x_ck": x_ck,
            "wq": wtile(Wq[hs].transpose(1, 0, 2).reshape(D, CW)),
            "wk": wtile(Wk[hs].transpose(1, 0, 2).reshape(D, CW)),
            "wv": wtile(Wv[hs].transpose(1, 0, 2).reshape(D, CW)),
            "wot": np.ascontiguousarray(
                wot.reshape(CW // 128, 128, D).transpose(1, 0, 2)),
            "masks": masks,
            "ones": np.ones((128, NST * NH), np.float32),
        })
    return in_maps


def kernel(x, Wq, Wk, Wv, W_O):
    x = np.asarray(x, np.float32)
    Wq = np.asarray(Wq, np.float32)
    Wk = np.asarray(Wk, np.float32)
    Wv = np.asarray(Wv, np.float32)
    W_O = np.asarray(W_O, np.float32)

    if "nc" not in _cache:
        _cache["nc"] = build_nc()
    nc = _cache["nc"]

    in_maps = shard_inputs(x, Wq, Wk, Wv, W_O)
    res = run_bass_kernel_spmd(nc, in_maps, core_ids=list(range(N_CORES)))
    _cache["last_results"] = res

    y = np.zeros((B, S, D), np.float32)
    for c in range(N_CORES):
        y[c // 2] += res.results[c]["yt"].T
    return y
